# revision 4
# baseline (speedup 1.0000x reference)
"""DeepJDOT loss kernel for 8 Trainium2 NeuronCores.

Math shortcut: the reference's greedy backtrack follows an optimal DTW path,
so sum(paths * dists) == acc[-1,-1] - dists[-1,-1] exactly. The kernel
therefore only needs the DTW cumulative-cost DP (no backtrack, no 268MB
paths tensor).

Sharding: source batch (64) split across 8 cores -> 8 sources x 64 targets
= 512 pairs per core.

Per-core device pipeline:
  1. PE: augmented matmul out[i,(t,j)] = 2*<xs_si,xt_tj> - |xt_tj|^2 - |xs_si|^2
     = -d2 (K=66: all-ones lhsT row pairs with -y2 rhs row; x2 lhsT row
     pairs with -1 rhs row; folding both sums into the matmul keeps every
     compute instruction at <=1 sync-wait, a walrus codegen limit).
  2. ACT: C = Sqrt(-out)  (= the pairwise L2 distance), per source
     slab [128 i, 64*128 (t,j)].
  3. DMA bounce through DRAM [i][pair][j] to transpose i <-> pair.
  4. DVE: DTW DP, 2 instructions per row over all 512 pairs at once:
       m   = min(prev, prev_shifted)               (up/diag candidates)
       row = tensor_tensor_scan(min, add)(m, C)    (adds left-neighbor chain)
     using a [4 blocks x (1 pad + 128)] interleaved free-dim layout whose
     INF pads are self-maintaining through the scan.
  5. dtw_cost = acc_end - C_end per pair -> [128, 4] output.

Host: tiny (64,64) entropic-OT sinkhorn + final contractions.
"""

import numpy as np

ALPHA = 1.0
BETA = 0.1
B_S, B_T = 64, 64
T_S, T_T = 128, 128
Q = 64
N_CORES = 8
S_PER_CORE = B_S // N_CORES          # 8 sources per core
NPAIR = S_PER_CORE * B_T             # 512 pairs per core
NBLK = NPAIR // 128                  # 4 partition blocks
K_AUG = Q + 2                        # 66 (extra -y2 row and -x2 row)
CHUNK = 512                          # matmul N per PSUM bank
NCHUNK = (B_T * T_T) // CHUNK        # 16 chunks per source slab
WROWS = 16                           # i-rows per C window
NWIN = T_S // WROWS                  # 8 windows
BW = T_T + 1                         # 129 slots per block (1 pad + 128)
FREE = NBLK * BW                     # 516 scan elements per row
INF = float(3e38)

_cache: dict = {}


def _build_module():
    import concourse.bass as bass
    import concourse.bacc as bacc
    import concourse.tile as tile
    from concourse import mybir

    f32 = mybir.dt.float32
    # Bacc (not plain Bass): its compile() pass legalizes multi-sem waits via
    # event semaphores; raw multi-wait instructions fail walrus codegen.
    nc = bacc.Bacc("TRN2", target_bir_lowering=False)

    # rhs cols [0, B_T*T_T) and lhs cols [B_T*T_T, +S_PER_CORE*T_S) packed in one
    # tensor so the first matmul carries a single DMA-sem wait (PE Matmult
    # allows only one sync-wait command in walrus codegen).
    LRW = B_T * T_T + S_PER_CORE * T_S
    lr_d = nc.dram_tensor("lr", [K_AUG, LRW], f32, kind="ExternalInput")
    dtw_d = nc.dram_tensor("dtw", [128, NBLK], f32, kind="ExternalOutput")
    # transpose bounce buffer [i][pair][j]
    cb = nc.dram_tensor("cbounce", [T_S, NPAIR, T_T], f32)

    with tile.TileContext(nc) as tc:
        with (
            tc.tile_pool(name="consts", bufs=1) as consts,
            tc.tile_pool(name="slabs", bufs=2) as slabs,
            tc.tile_pool(name="wins", bufs=2) as wins,
            tc.tile_pool(name="accs", bufs=1) as accs,
            tc.tile_pool(name="ms", bufs=2) as ms,
            tc.tile_pool(name="outs", bufs=1) as outs,
            tc.tile_pool(name="psum", bufs=4, space="PSUM") as psum,
        ):
            # ---- load constants ----
            lr_sb = consts.tile([K_AUG, LRW], f32)
            nc.sync.dma_start(out=lr_sb[:], in_=lr_d[:])
            LOFF = B_T * T_T

            # ---- phase A: C slabs per source -> DRAM bounce ----
            for s in range(S_PER_CORE):
                slab = slabs.tile([T_S, B_T * T_T], f32)
                for k in range(NCHUNK):
                    pt = psum.tile([T_S, CHUNK], f32)
                    nc.tensor.matmul(
                        pt[:],
                        lr_sb[:, LOFF + s * T_S:LOFF + (s + 1) * T_S],
                        lr_sb[:, k * CHUNK:(k + 1) * CHUNK],
                        start=True, stop=True,
                    )
                    nc.scalar.activation(
                        out=slab[:, k * CHUNK:(k + 1) * CHUNK],
                        in_=pt[:],
                        func=mybir.ActivationFunctionType.Sqrt,
                        scale=-1.0,
                    )
                nc.sync.dma_start(
                    out=cb[:, s * B_T:(s + 1) * B_T, :],
                    in_=slab[:].rearrange("i (t j) -> i t j", t=B_T),
                )

            # ---- phase B: DTW DP ----
            # acc buffers: [128, 1 + FREE]; slot layout (s-space):
            #   s=0 global pad; block b: values at 1+b*BW+j (j<128), trailpad at 1+b*BW+128
            acc_init = accs.tile([128, 1 + FREE], f32)
            accA = accs.tile([128, 1 + FREE], f32)
            accB = accs.tile([128, 1 + FREE], f32)
            nc.vector.memset(acc_init[:], INF)
            for b in range(NBLK):
                # virtual corner acc[-1,-1] = 0 feeding each block's (0,0)
                nc.vector.memset(acc_init[:, b * BW:b * BW + 1], 0.0)
            nc.vector.memset(accA[:, 0:1], INF)
            nc.vector.memset(accB[:, 0:1], INF)

            cbap = cb[:]
            st_i, st_p = NPAIR * T_T, T_T
            win_tiles = []
            cur = None
            for i in range(T_S):
                w, r = divmod(i, WROWS)
                if r == 0:
                    win = wins.tile([128, WROWS, FREE], f32)
                    win_tiles.append(win)
                    for b in range(NBLK):
                        # INF pad column at slot b*BW+128 (self-resets scan state)
                        nc.vector.memset(win[:, :, b * BW + T_T:b * BW + T_T + 1], INF)
                        src = bass.AP(
                            tensor=cbap.tensor,
                            offset=cbap.offset + w * WROWS * st_i + b * 128 * st_p,
                            ap=[[st_p, 128], [st_i, WROWS], [1, T_T]],
                        )
                        nc.sync.dma_start(out=win[:, :, b * BW:b * BW + T_T], in_=src)
                prev = acc_init if i == 0 else (accA if i % 2 == 1 else accB)
                cur = accA if i % 2 == 0 else accB
                m = ms.tile([128, FREE], f32)
                nc.vector.tensor_tensor(
                    out=m[:], in0=prev[:, 1:1 + FREE], in1=prev[:, 0:FREE],
                    op=mybir.AluOpType.min,
                )
                nc.vector.tensor_tensor_scan(
                    out=cur[:, 1:1 + FREE], data0=m[:], data1=win[:, r, :],
                    initial=INF, op0=mybir.AluOpType.min, op1=mybir.AluOpType.add,
                )

            # ---- phase C: dtw = acc_end - C_end ----
            dtw_sb = outs.tile([128, NBLK], f32)
            acc_ap = cur[:]
            acc_end = bass.AP(tensor=acc_ap.tensor, offset=acc_ap.offset + 1 + T_T - 1,
                              ap=[list(acc_ap.ap[0]), [BW, NBLK]])
            lastwin = win_tiles[-1][:]
            c_end = bass.AP(tensor=lastwin.tensor,
                            offset=lastwin.offset + (WROWS - 1) * FREE + T_T - 1,
                            ap=[list(lastwin.ap[0]), [BW, NBLK]])
            nc.vector.tensor_tensor(out=dtw_sb[:], in0=acc_end, in1=c_end,
                                    op=mybir.AluOpType.subtract)
            nc.sync.dma_start(out=dtw_d[:], in_=dtw_sb[:])

    nc.finalize()
    return nc


def _get_module():
    if "nc" not in _cache:
        _cache["nc"] = _build_module()
    return _cache["nc"]


def _prep_core_inputs(xs_all, xt_all):
    """Per-core input dict list. xs_all (64,128,64), xt_all (64,128,64) f32."""
    y2 = (xt_all * xt_all).sum(-1)                       # (64,128)
    NRHS = B_T * T_T
    in_maps = []
    for c in range(N_CORES):
        xs = xs_all[c * S_PER_CORE:(c + 1) * S_PER_CORE]  # (8,128,64)
        lr = np.empty((K_AUG, NRHS + S_PER_CORE * T_S), np.float32)
        lr[:Q, :NRHS] = xt_all.reshape(NRHS, Q).T
        lr[Q, :NRHS] = -y2.reshape(-1)
        lr[Q + 1, :NRHS] = -1.0
        lr[:Q, NRHS:] = (2.0 * xs).reshape(S_PER_CORE * T_S, Q).T
        lr[Q, NRHS:] = 1.0
        lr[Q + 1, NRHS:] = (xs * xs).sum(-1).reshape(-1)
        in_maps.append({"lr": lr})
    return in_maps


def _logsumexp(x, axis):
    m = np.max(x, axis=axis, keepdims=True)
    return (m + np.log(np.sum(np.exp(x - m), axis=axis, keepdims=True))).squeeze(axis)


def _sinkhorn(M, iters=300, eps_scale=0.02):
    M = M.astype(np.float64)
    eps = eps_scale * M.max()
    loga = np.log(np.full(M.shape[0], 1.0 / M.shape[0]))
    logb = np.log(np.full(M.shape[1], 1.0 / M.shape[1]))
    f = np.zeros(M.shape[0])
    g = np.zeros(M.shape[1])
    for _ in range(iters):
        f = eps * loga - eps * _logsumexp((g[None, :] - M) / eps, axis=1)
        g = eps * logb - eps * _logsumexp((f[:, None] - M) / eps, axis=0)
    return np.exp((f[:, None] + g[None, :] - M) / eps)


def _postprocess(dtw_cost, similarity_CE):
    sim = np.asarray(similarity_CE, np.float32)
    cost_OT = ALPHA * dtw_cost + BETA * sim
    gamma = _sinkhorn(cost_OT)
    alpha_cost = float((gamma * dtw_cost).sum())
    beta_cost = float((gamma * sim.astype(np.float64)).sum())
    length = (Q + Q) / 2.0
    return (
        np.float32(ALPHA * alpha_cost / length),
        np.float32(BETA * beta_cost / length),
        gamma.astype(np.float32),
    )


def kernel(out_conv_source, out_conv_target, labels_source, similarity_CE):
    from concourse.bass_utils import run_bass_kernel_spmd

    xs_all = np.asarray(out_conv_source, np.float32)
    xt_all = np.asarray(out_conv_target, np.float32)
    in_maps = _prep_core_inputs(xs_all, xt_all)
    nc = _get_module()
    res = run_bass_kernel_spmd(nc, in_maps, core_ids=list(range(N_CORES)))
    dtw_rows = []
    for c in range(N_CORES):
        out = np.asarray(res.results[c]["dtw"])          # [128, 4]
        dtw_rows.append(out.T.reshape(S_PER_CORE, B_T))  # pair = b*128+p = s*64+t
    dtw_cost = np.concatenate(dtw_rows, 0).astype(np.float64)  # (64, 64)
    return _postprocess(dtw_cost, similarity_CE)


# revision 5
# speedup vs baseline: 1.2252x; 1.2252x over previous
"""DeepJDOT loss kernel for 8 Trainium2 NeuronCores.

Math shortcut: the reference's greedy backtrack follows an optimal DTW path,
so sum(paths * dists) == acc[-1,-1] - dists[-1,-1] exactly. The kernel
therefore only needs the DTW cumulative-cost DP (no backtrack, no 268MB
paths tensor).

Sharding: source batch (64) split across 8 cores -> 8 sources x 64 targets
= 512 pairs per core.

Per-core device pipeline:
  1. PE: augmented matmul out[i,(t,j)] = 2*<xs_si,xt_tj> - |xt_tj|^2 - |xs_si|^2
     = -d2, in bf16 (4x faster than f32 on PE). K=68: the -y2 and -x2 sums
     ride along as bf16 hi+lo compensated pairs (rows 64..67), so only the
     2<xs,xt> term carries bf16 rounding (~1e-3 abs on C). Folding the sums
     into the matmul also keeps every compute instruction at <=1 sync-wait,
     a walrus codegen limit.
  2. ACT: C = Sqrt(-out)  (= the pairwise L2 distance), per source
     slab [128 i, 64*128 (t,j)].
  3. DMA bounce through DRAM [i][pair][j] to transpose i <-> pair.
  4. DVE: DTW DP, 2 instructions per row over all 512 pairs at once:
       m   = min(prev, prev_shifted)               (up/diag candidates)
       row = tensor_tensor_scan(min, add)(m, C)    (adds left-neighbor chain)
     using a [4 blocks x (1 pad + 128)] interleaved free-dim layout whose
     INF pads are self-maintaining through the scan.
  5. dtw_cost = acc_end - C_end per pair -> [128, 4] output.

Host: tiny (64,64) entropic-OT sinkhorn + final contractions.
"""

import numpy as np

ALPHA = 1.0
BETA = 0.1
B_S, B_T = 64, 64
T_S, T_T = 128, 128
Q = 64
N_CORES = 8
S_PER_CORE = B_S // N_CORES          # 8 sources per core
NPAIR = S_PER_CORE * B_T             # 512 pairs per core
NBLK = NPAIR // 128                  # 4 partition blocks
K_AUG = Q + 4                        # 68 (hi/lo rows for -y2 and -x2)
CHUNK = 512                          # matmul N per PSUM bank
NCHUNK = (B_T * T_T) // CHUNK        # 16 chunks per source slab
WROWS = 16                           # i-rows per C window
NWIN = T_S // WROWS                  # 8 windows
BW = T_T + 1                         # 129 slots per block (1 pad + 128)
FREE = NBLK * BW                     # 516 scan elements per row
INF = float(3e38)

_cache: dict = {}


def _build_module():
    import concourse.bass as bass
    import concourse.bacc as bacc
    import concourse.tile as tile
    from concourse import mybir

    f32 = mybir.dt.float32
    # Bacc (not plain Bass): its compile() pass legalizes multi-sem waits via
    # event semaphores; raw multi-wait instructions fail walrus codegen.
    nc = bacc.Bacc("TRN2", target_bir_lowering=False)

    # rhs cols [0, B_T*T_T) and lhs cols [B_T*T_T, +S_PER_CORE*T_S) packed in one
    # tensor so the first matmul carries a single DMA-sem wait (PE Matmult
    # allows only one sync-wait command in walrus codegen).
    bf16 = mybir.dt.bfloat16
    LRW = B_T * T_T + S_PER_CORE * T_S
    lr_d = nc.dram_tensor("lr", [K_AUG, LRW], bf16, kind="ExternalInput")
    dtw_d = nc.dram_tensor("dtw", [128, NBLK], f32, kind="ExternalOutput")
    # transpose bounce buffer [pair][i][j] (window reads get 8KB-contiguous runs)
    cb = nc.dram_tensor("cbounce", [NPAIR, T_S, T_T], f32)

    with tile.TileContext(nc) as tc:
        with (
            tc.tile_pool(name="consts", bufs=1) as consts,
            tc.tile_pool(name="slabs", bufs=2) as slabs,
            tc.tile_pool(name="wins", bufs=2) as wins,
            tc.tile_pool(name="accs", bufs=1) as accs,
            tc.tile_pool(name="ms", bufs=2) as ms,
            tc.tile_pool(name="outs", bufs=1) as outs,
            tc.tile_pool(name="psum", bufs=4, space="PSUM") as psum,
        ):
            # ---- load constants ----
            lr_sb = consts.tile([K_AUG, LRW], bf16)
            nc.sync.dma_start(out=lr_sb[:], in_=lr_d[:])
            LOFF = B_T * T_T

            # ---- phase A: C slabs per source -> DRAM bounce ----
            for s in range(S_PER_CORE):
                slab = slabs.tile([T_S, B_T * T_T], f32)
                for k in range(NCHUNK):
                    pt = psum.tile([T_S, CHUNK], f32)
                    nc.tensor.matmul(
                        pt[:],
                        lr_sb[:, LOFF + s * T_S:LOFF + (s + 1) * T_S],
                        lr_sb[:, k * CHUNK:(k + 1) * CHUNK],
                        start=True, stop=True,
                    )
                    nc.scalar.activation(
                        out=slab[:, k * CHUNK:(k + 1) * CHUNK],
                        in_=pt[:],
                        func=mybir.ActivationFunctionType.Sqrt,
                        scale=-1.0,
                    )
                # dst iterated (i, t, j) to match the slab's element order
                cba = cb[:]
                dst = bass.AP(
                    tensor=cba.tensor,
                    offset=cba.offset + s * B_T * T_S * T_T,
                    ap=[[T_T, T_S], [T_S * T_T, B_T], [1, T_T]],
                )
                nc.sync.dma_start(
                    out=dst,
                    in_=slab[:].rearrange("i (t j) -> i t j", t=B_T),
                )

            # ---- phase B: DTW DP ----
            # acc buffers: [128, 1 + FREE]; slot layout (s-space):
            #   s=0 global pad; block b: values at 1+b*BW+j (j<128), trailpad at 1+b*BW+128
            acc_init = accs.tile([128, 1 + FREE], f32)
            accA = accs.tile([128, 1 + FREE], f32)
            accB = accs.tile([128, 1 + FREE], f32)
            nc.vector.memset(acc_init[:], INF)
            for b in range(NBLK):
                # virtual corner acc[-1,-1] = 0 feeding each block's (0,0)
                nc.vector.memset(acc_init[:, b * BW:b * BW + 1], 0.0)
            nc.vector.memset(accA[:, 0:1], INF)
            nc.vector.memset(accB[:, 0:1], INF)

            cbap = cb[:]
            st_p, st_i = T_S * T_T, T_T
            win_tiles = []
            cur = None
            for i in range(T_S):
                w, r = divmod(i, WROWS)
                if r == 0:
                    win = wins.tile([128, WROWS, FREE], f32)
                    win_tiles.append(win)
                    for b in range(NBLK):
                        # INF pad column at slot b*BW+128 (self-resets scan state)
                        nc.vector.memset(win[:, :, b * BW + T_T:b * BW + T_T + 1], INF)
                        # (rw, j) merge into one 8KB-contiguous run per pair
                        src = bass.AP(
                            tensor=cbap.tensor,
                            offset=cbap.offset + w * WROWS * st_i + b * 128 * st_p,
                            ap=[[st_p, 128], [1, WROWS * T_T]],
                        )
                        nc.sync.dma_start(out=win[:, :, b * BW:b * BW + T_T], in_=src)
                prev = acc_init if i == 0 else (accA if i % 2 == 1 else accB)
                cur = accA if i % 2 == 0 else accB
                m = ms.tile([128, FREE], f32)
                nc.vector.tensor_tensor(
                    out=m[:], in0=prev[:, 1:1 + FREE], in1=prev[:, 0:FREE],
                    op=mybir.AluOpType.min,
                )
                nc.vector.tensor_tensor_scan(
                    out=cur[:, 1:1 + FREE], data0=m[:], data1=win[:, r, :],
                    initial=INF, op0=mybir.AluOpType.min, op1=mybir.AluOpType.add,
                )

            # ---- phase C: dtw = acc_end - C_end ----
            dtw_sb = outs.tile([128, NBLK], f32)
            acc_ap = cur[:]
            acc_end = bass.AP(tensor=acc_ap.tensor, offset=acc_ap.offset + 1 + T_T - 1,
                              ap=[list(acc_ap.ap[0]), [BW, NBLK]])
            lastwin = win_tiles[-1][:]
            c_end = bass.AP(tensor=lastwin.tensor,
                            offset=lastwin.offset + (WROWS - 1) * FREE + T_T - 1,
                            ap=[list(lastwin.ap[0]), [BW, NBLK]])
            nc.vector.tensor_tensor(out=dtw_sb[:], in0=acc_end, in1=c_end,
                                    op=mybir.AluOpType.subtract)
            nc.sync.dma_start(out=dtw_d[:], in_=dtw_sb[:])

    nc.finalize()
    return nc


def _get_module():
    if "nc" not in _cache:
        _cache["nc"] = _build_module()
    return _cache["nc"]


def _hilo(v):
    """Split f32 vector into compensated bf16 (hi, lo) pair."""
    import ml_dtypes
    hi = v.astype(ml_dtypes.bfloat16)
    lo = (v - hi.astype(np.float32)).astype(ml_dtypes.bfloat16)
    return hi, lo


def _prep_core_inputs(xs_all, xt_all):
    """Per-core input dict list. xs_all (64,128,64), xt_all (64,128,64) f32."""
    import ml_dtypes
    bf16 = ml_dtypes.bfloat16
    y2 = (xt_all.astype(np.float64) ** 2).sum(-1).astype(np.float32)  # (64,128)
    y2hi, y2lo = _hilo(-y2.reshape(-1))
    NRHS = B_T * T_T
    in_maps = []
    for c in range(N_CORES):
        xs = xs_all[c * S_PER_CORE:(c + 1) * S_PER_CORE]  # (8,128,64)
        x2 = (xs.astype(np.float64) ** 2).sum(-1).astype(np.float32)
        x2hi, x2lo = _hilo(x2.reshape(-1))
        lr = np.empty((K_AUG, NRHS + S_PER_CORE * T_S), bf16)
        lr[:Q, :NRHS] = xt_all.reshape(NRHS, Q).T.astype(bf16)
        lr[Q, :NRHS] = y2hi
        lr[Q + 1, :NRHS] = y2lo
        lr[Q + 2, :NRHS] = bf16(-1.0)
        lr[Q + 3, :NRHS] = bf16(-1.0)
        lr[:Q, NRHS:] = (2.0 * xs).reshape(S_PER_CORE * T_S, Q).T.astype(bf16)
        lr[Q, NRHS:] = bf16(1.0)
        lr[Q + 1, NRHS:] = bf16(1.0)
        lr[Q + 2, NRHS:] = x2hi
        lr[Q + 3, NRHS:] = x2lo
        in_maps.append({"lr": lr})
    return in_maps


def _logsumexp(x, axis):
    m = np.max(x, axis=axis, keepdims=True)
    return (m + np.log(np.sum(np.exp(x - m), axis=axis, keepdims=True))).squeeze(axis)


def _sinkhorn(M, iters=300, eps_scale=0.02):
    M = M.astype(np.float64)
    eps = eps_scale * M.max()
    loga = np.log(np.full(M.shape[0], 1.0 / M.shape[0]))
    logb = np.log(np.full(M.shape[1], 1.0 / M.shape[1]))
    f = np.zeros(M.shape[0])
    g = np.zeros(M.shape[1])
    for _ in range(iters):
        f = eps * loga - eps * _logsumexp((g[None, :] - M) / eps, axis=1)
        g = eps * logb - eps * _logsumexp((f[:, None] - M) / eps, axis=0)
    return np.exp((f[:, None] + g[None, :] - M) / eps)


def _postprocess(dtw_cost, similarity_CE):
    sim = np.asarray(similarity_CE, np.float32)
    cost_OT = ALPHA * dtw_cost + BETA * sim
    gamma = _sinkhorn(cost_OT)
    alpha_cost = float((gamma * dtw_cost).sum())
    beta_cost = float((gamma * sim.astype(np.float64)).sum())
    length = (Q + Q) / 2.0
    return (
        np.float32(ALPHA * alpha_cost / length),
        np.float32(BETA * beta_cost / length),
        gamma.astype(np.float32),
    )


def kernel(out_conv_source, out_conv_target, labels_source, similarity_CE):
    from concourse.bass_utils import run_bass_kernel_spmd

    xs_all = np.asarray(out_conv_source, np.float32)
    xt_all = np.asarray(out_conv_target, np.float32)
    in_maps = _prep_core_inputs(xs_all, xt_all)
    nc = _get_module()
    res = run_bass_kernel_spmd(nc, in_maps, core_ids=list(range(N_CORES)))
    dtw_rows = []
    for c in range(N_CORES):
        out = np.asarray(res.results[c]["dtw"])          # [128, 4]
        dtw_rows.append(out.T.reshape(S_PER_CORE, B_T))  # pair = b*128+p = s*64+t
    dtw_cost = np.concatenate(dtw_rows, 0).astype(np.float64)  # (64, 64)
    return _postprocess(dtw_cost, similarity_CE)
